# revision 1
# baseline (speedup 1.0000x reference)
"""Per-pixel dynamic 5x5 conv (KernelConv) on 8 Trainium2 NeuronCores.

out[b,c,h,w] = sum_{i,j} core[b,(i*5+j)*C+c,h,w] * pad(data)[b,c,h+i,w+j]

Sharding: channel groups of 8 per core (x 4 batches = 32 channel-images/core).
Layout on chip: partitions = h (128), free dim = image-blocks x w.
The host pre-pads data in h and w; the kernel loads 5 row-shifted copies of
the padded data (row shifts cannot be done on-chip: engine/DMA SBUF access
patterns may only start at partitions 0/32/64/96). All 25 taps then become
full-128-partition tensor_tensor ops with a column offset on the data AP.

fp32 exact: DVE computes the 25 products; the 24 accumulate-adds are split
between DVE and GpSimd (concurrent: 1-port DVE tensor_tensor never contends
with GpSimd for the shared SBUF port) via two accumulator chains.
"""

import numpy as np

B, C, H, W = 4, 64, 128, 128
K, PAD, KK = 5, 2, 25
NCORES = 8
CPC = C // NCORES            # channels per core = 8
NIMG = B * CPC               # channel-images per core = 32
GRP = 16                     # images per compute group
NG = NIMG // GRP             # groups = 2
WP = W + 2 * PAD             # 132
HP = H + 2 * PAD             # 132
KCH = 1                      # k-slices per core-load DMA
CORE_BUFS = 4                # core tile pool depth
DATA_ENG = "scalar"          # HWDGE ring for data/out DMAs: sync (SP) or scalar (ACT)

# k's whose accumulate-add runs on GpSimd (spread through the k order so the
# GpSimd chain is fed continuously). k%3==0 stays on DVE -> 9 DVE / 16 GpSimd.
GPSIMD_KS = frozenset(k for k in range(KK) if k % 3 != 0)

_CACHE = {}


def _build_module(debug=False):
    import concourse.tile as tile
    from concourse import bacc, mybir

    f32 = mybir.dt.float32
    nc = bacc.Bacc(
        "TRN2", target_bir_lowering=False, debug=debug, num_devices=NCORES
    )
    core_d = nc.dram_tensor(
        "core", [KK, H, NIMG * W], f32, kind="ExternalInput"
    ).ap()
    data_d = nc.dram_tensor(
        "data", [HP, NIMG * WP], f32, kind="ExternalInput"
    ).ap()
    out_d = nc.dram_tensor(
        "out", [H, NIMG * W], f32, kind="ExternalOutput"
    ).ap()

    with tile.TileContext(nc) as tc:
        with (
            tc.tile_pool(name="shifts", bufs=2) as sh_pool,
            tc.tile_pool(name="corep", bufs=CORE_BUFS) as c_pool,
            tc.tile_pool(name="prodp", bufs=4) as p_pool,
            tc.tile_pool(name="accvp", bufs=2) as av_pool,
            tc.tile_pool(name="accgp", bufs=2) as ag_pool,
        ):
            for g in range(NG):
                gw = slice(g * GRP * W, (g + 1) * GRP * W)
                gwp = slice(g * GRP * WP, (g + 1) * GRP * WP)
                sh = sh_pool.tile([H, K * GRP * WP], f32, tag="sh")
                sh4 = sh.rearrange("p (s b w) -> p s b w", s=K, b=GRP)
                for i in range(K):
                    # contiguous 2D APs on both sides: fewest DMA descriptors
                    # (3D APs here overflowed walrus's 16-bit IO-DGE
                    # semaphore_wait_value field)
                    getattr(nc, DATA_ENG).dma_start(
                        sh[:, i * GRP * WP : (i + 1) * GRP * WP],
                        data_d[i : i + H, gwp],
                    )
                acc_v = av_pool.tile([H, GRP * W], f32, tag="accv")
                acc_g = ag_pool.tile([H, GRP * W], f32, tag="accg")
                av3 = acc_v.rearrange("p (b w) -> p b w", b=GRP)
                ag3 = acc_g.rearrange("p (b w) -> p b w", b=GRP)
                first_v, first_g = True, True
                cts = {}
                for kc in range(0, KK, KCH):
                    kn = min(KCH, KK - kc)
                    ct = c_pool.tile([H, KCH * GRP * W], f32, tag="ct")
                    # KCH=1 measured fastest: DMA occupancy is byte-bound with
                    # no per-transfer fixed cost, and finer tiles pipeline better.
                    nc.sync.dma_start(
                        ct[:, : kn * GRP * W],
                        core_d[kc : kc + kn, :, gw].rearrange("k p w -> p k w"),
                    )
                    for dk in range(kn):
                        cts[kc + dk] = ct[:, dk * GRP * W : (dk + 1) * GRP * W]
                for k in range(KK):
                    i, j = divmod(k, K)
                    ct3 = cts[k].rearrange("p (b w) -> p b w", b=GRP)
                    din = sh4[:, i, :, j : j + W]
                    if k in GPSIMD_KS:
                        if first_g:
                            nc.vector.tensor_mul(ag3, ct3, din)
                            first_g = False
                        else:
                            pt = p_pool.tile([H, GRP * W], f32, tag="pt")
                            pt3 = pt.rearrange("p (b w) -> p b w", b=GRP)
                            nc.vector.tensor_mul(pt3, ct3, din)
                            nc.gpsimd.tensor_add(ag3, ag3, pt3)
                    else:
                        if first_v:
                            nc.vector.tensor_mul(av3, ct3, din)
                            first_v = False
                        else:
                            pt = p_pool.tile([H, GRP * W], f32, tag="pt")
                            pt3 = pt.rearrange("p (b w) -> p b w", b=GRP)
                            nc.vector.tensor_mul(pt3, ct3, din)
                            nc.vector.tensor_add(av3, av3, pt3)
                nc.vector.tensor_add(av3, av3, ag3)
                getattr(nc, DATA_ENG).dma_start(out_d[:, gw], acc_v[:])

    nc.compile()
    return nc


def get_nc(debug=False):
    key = ("nc", debug)
    if key not in _CACHE:
        _CACHE[key] = _build_module(debug=debug)
    return _CACHE[key]


def prep_inputs(data, core):
    """Full inputs -> list of per-core input dicts (host-side shard + pad)."""
    data = np.ascontiguousarray(data, dtype=np.float32)
    core = np.ascontiguousarray(core, dtype=np.float32)
    core_t = np.ascontiguousarray(
        core.reshape(B, KK, C, H, W).transpose(1, 3, 0, 2, 4)
    )  # [KK, H, B, C, W]
    dp = np.zeros((HP, B, C, WP), np.float32)
    dp[PAD : PAD + H, :, :, PAD : PAD + W] = data.transpose(2, 0, 1, 3)
    in_maps = []
    for r in range(NCORES):
        cs = slice(r * CPC, (r + 1) * CPC)
        core_r = np.ascontiguousarray(core_t[:, :, :, cs, :]).reshape(
            KK, H, NIMG * W
        )
        data_r = np.ascontiguousarray(dp[:, :, cs, :]).reshape(HP, NIMG * WP)
        in_maps.append({"core": core_r, "data": data_r})
    return in_maps


def assemble(per_core_outs):
    """Per-core 'out' arrays [H, NIMG*W] -> full [B, C, H, W]."""
    out = np.empty((B, C, H, W), np.float32)
    for r, o in enumerate(per_core_outs):
        cs = slice(r * CPC, (r + 1) * CPC)
        out[:, cs] = o.reshape(H, B, CPC, W).transpose(1, 2, 0, 3)
    return out


def run_spmd(in_maps, trace=False, trace_cores=None):
    from concourse.bass_utils import run_bass_kernel_spmd

    return run_bass_kernel_spmd(
        get_nc(),
        in_maps,
        list(range(NCORES)),
        trace=trace,
        trace_cores=trace_cores,
    )


def kernel(data, core):
    res = run_spmd(prep_inputs(data, core))
    return assemble([res.results[r]["out"] for r in range(NCORES)])



# revision 2
# speedup vs baseline: 1.7003x; 1.7003x over previous
"""Per-pixel dynamic 5x5 conv (KernelConv) on 8 Trainium2 NeuronCores.

out[b,c,h,w] = sum_{i,j} core[b,(i*5+j)*C+c,h,w] * pad(data)[b,c,h+i,w+j]

Sharding: channel groups of 8 per core (x 4 batches = 32 channel-images/core).
Layout on chip: partitions = h (128), free dim = image-blocks x w.

All HBM traffic is bf16 (host converts): halves the DMA bytes vs f32, and the
2e-2 rel-err budget dwarfs bf16 rounding (~0.5% worst case here).

Compute split so no engine exceeds the ~91us bf16 DMA roofline:
- DVE does ONLY the 25 products, batched 5 taps per instruction (all-bf16
  operands hit the DVE 2x perf mode; a mixed f32 output would forfeit it).
  Host stores core k-slices j-major (k' = j*5+i) so one instruction pairs
  core chunk j with all 5 row-shift copies via a 3-free-dim AP — no
  overlapping j-window needed.
- The 24 per-pixel accumulate-adds run on the otherwise-idle TensorEngine as
  identity matmuls accumulating into PSUM (f32, exact): 25 matmuls per
  512-col bank chunk. ACT (scalar engine) drains PSUM -> bf16 SBUF for the
  output DMA. GpSimd stays idle (Pool adds cost ~4x DVE in the cost model).

The host pre-pads data in h and w; the kernel loads 5 row-shifted copies of
the padded data (row shifts cannot be done on-chip: engine/DMA SBUF access
patterns may only start at partitions 0/32/64/96).
"""

import numpy as np
import ml_dtypes

B, C, H, W = 4, 64, 128, 128
K, PAD, KK = 5, 2, 25
NCORES = 8
CPC = C // NCORES            # channels per core = 8
NIMG = B * CPC               # channel-images per core = 32
GRP = 16                     # images per compute group
NG = NIMG // GRP             # groups = 2
WP = W + 2 * PAD             # 132
HP = H + 2 * PAD             # 132
GW = GRP * W                 # 2048 free columns per group
CHUNK = 512                  # PSUM bank = 512 f32 per partition
NCH = GW // CHUNK            # 4 bank chunks per group

BF16 = ml_dtypes.bfloat16

_CACHE = {}


def _build_module(debug=False):
    import concourse.tile as tile
    from concourse import bacc, bass, mybir

    f32 = mybir.dt.float32
    bf16 = mybir.dt.bfloat16
    nc = bacc.Bacc(
        "TRN2", target_bir_lowering=False, debug=debug, num_devices=NCORES
    )
    # core k-slices stored j-major: slice k' = j*K + i holds tap (i, j)
    core_d = nc.dram_tensor(
        "core", [KK, H, NIMG * W], bf16, kind="ExternalInput"
    ).ap()
    data_d = nc.dram_tensor(
        "data", [HP, NIMG * WP], bf16, kind="ExternalInput"
    ).ap()
    ident_d = nc.dram_tensor("ident", [128, 128], bf16, kind="ExternalInput").ap()
    out_d = nc.dram_tensor("out", [H, NIMG * W], bf16, kind="ExternalOutput").ap()

    with tile.TileContext(nc) as tc:
        with (
            tc.tile_pool(name="constp", bufs=1) as k_pool,
            tc.tile_pool(name="shifts", bufs=NG) as sh_pool,
            tc.tile_pool(name="corep", bufs=3) as c_pool,
            tc.tile_pool(name="prodp", bufs=2) as p_pool,
            tc.tile_pool(name="outp", bufs=2) as o_pool,
            tc.tile_pool(
                name="psump", bufs=NG, space=bass.MemorySpace.PSUM
            ) as ps_pool,
        ):
            ident = k_pool.tile([128, 128], bf16, tag="ident")
            nc.scalar.dma_start(ident[:], ident_d[:, :])
            # all shift copies up front so nothing on the scalar ring blocks
            # a later group's data loads behind a PSUM-drain dependency
            shs = []
            for g in range(NG):
                gwp = slice(g * GRP * WP, (g + 1) * GRP * WP)
                sh = sh_pool.tile([H, K * GRP * WP], bf16, tag="sh", name=f"sh{g}")
                for i in range(K):
                    # contiguous 2D APs on both sides: fewest DMA descriptors
                    # (3D APs overflow walrus's 16-bit IO-DGE
                    # semaphore_wait_value field)
                    nc.scalar.dma_start(
                        sh[:, i * GRP * WP : (i + 1) * GRP * WP],
                        data_d[i : i + H, gwp],
                    )
                shs.append(sh.rearrange("p (s b w) -> p s b w", s=K, b=GRP))
            for g in range(NG):
                gw = slice(g * GW, (g + 1) * GW)
                sh4 = shs[g]
                ps = ps_pool.tile([H, GW], f32, tag="ps", name=f"ps{g}")
                for j in range(K):
                    ct = c_pool.tile([H, K * GW], bf16, tag="ct")
                    for i in range(K):
                        nc.sync.dma_start(
                            ct[:, i * GW : (i + 1) * GW],
                            core_d[j * K + i, :, gw],
                        )
                    pt = p_pool.tile([H, K * GW], bf16, tag="pt")
                    ct4 = ct.rearrange("p (s b w) -> p s b w", s=K, b=GRP)
                    pt4 = pt.rearrange("p (s b w) -> p s b w", s=K, b=GRP)
                    nc.vector.tensor_mul(pt4, ct4, sh4[:, :, :, j : j + W])
                    for i in range(K):
                        for c in range(NCH):
                            off = i * GW + c * CHUNK
                            nc.tensor.matmul(
                                ps[:, c * CHUNK : (c + 1) * CHUNK],
                                ident[:],
                                pt[:, off : off + CHUNK],
                                start=(j == 0 and i == 0),
                                stop=(j == K - 1 and i == K - 1),
                            )
                ot = o_pool.tile([H, GW], bf16, tag="ot")
                nc.scalar.copy(ot[:], ps[:])
                nc.scalar.dma_start(out_d[:, gw], ot[:])

    nc.compile()
    return nc


def get_nc(debug=False):
    key = ("nc", debug)
    if key not in _CACHE:
        _CACHE[key] = _build_module(debug=debug)
    return _CACHE[key]


def prep_inputs(data, core):
    """Full inputs -> list of per-core input dicts (host-side shard + pad)."""
    data = np.asarray(data, dtype=np.float32)
    core = np.asarray(core, dtype=np.float32)
    # [b, i, j, c, h, w] -> [j, i, h, b, c, w]: k-slices j-major on device
    core_t = np.ascontiguousarray(
        core.reshape(B, K, K, C, H, W).transpose(2, 1, 4, 0, 3, 5).astype(BF16)
    )
    dp = np.zeros((HP, B, C, WP), BF16)
    dp[PAD : PAD + H, :, :, PAD : PAD + W] = data.transpose(2, 0, 1, 3)
    ident = np.ascontiguousarray(np.eye(128, dtype=BF16))
    in_maps = []
    for r in range(NCORES):
        cs = slice(r * CPC, (r + 1) * CPC)
        core_r = np.ascontiguousarray(core_t[:, :, :, :, cs, :]).reshape(
            KK, H, NIMG * W
        )
        data_r = np.ascontiguousarray(dp[:, :, cs, :]).reshape(HP, NIMG * WP)
        in_maps.append({"core": core_r, "data": data_r, "ident": ident})
    return in_maps


def assemble(per_core_outs):
    """Per-core 'out' arrays [H, NIMG*W] (bf16) -> full [B, C, H, W] f32."""
    out = np.empty((B, C, H, W), np.float32)
    for r, o in enumerate(per_core_outs):
        cs = slice(r * CPC, (r + 1) * CPC)
        out[:, cs] = (
            np.asarray(o).astype(np.float32)
            .reshape(H, B, CPC, W)
            .transpose(1, 2, 0, 3)
        )
    return out


def run_spmd(in_maps, trace=False, trace_cores=None):
    from concourse.bass_utils import run_bass_kernel_spmd

    return run_bass_kernel_spmd(
        get_nc(),
        in_maps,
        list(range(NCORES)),
        trace=trace,
        trace_cores=trace_cores,
    )


def kernel(data, core):
    res = run_spmd(prep_inputs(data, core))
    return assemble([res.results[r]["out"] for r in range(NCORES)])


# revision 4
# speedup vs baseline: 1.7994x; 1.0583x over previous
"""Per-pixel dynamic 5x5 conv (KernelConv) on 8 Trainium2 NeuronCores.

out[b,c,h,w] = sum_{i,j} core[b,(i*5+j)*C+c,h,w] * pad(data)[b,c,h+i,w+j]

Sharding: channel groups of 8 per core (x 4 batches = 32 channel-images/core).
Layout on chip: partitions = h (128), free dim = image-blocks x w.

All HBM traffic is bf16 (host converts): halves the DMA bytes vs f32, and the
2e-2 rel-err budget dwarfs bf16 rounding (~0.5% worst case here).

Compute split so no engine exceeds the ~91us bf16 DMA roofline:
- DVE does the 25 products, batched 5 taps per instruction for j<4 (all-bf16
  operands hit the DVE 2x perf mode; a mixed f32 output would forfeit it).
  Host stores core k-slices j-major (k' = j*5+i) so one instruction pairs
  core chunk j with all 5 row-shift copies via a 3-free-dim AP.
- Taps j=0..3 accumulate on the otherwise-idle TensorEngine as identity
  matmuls into PSUM (f32, exact); ACT drains PSUM -> bf16 SBUF as soon as
  the 20th tap lands.
- Taps j=4 accumulate on DVE (per-k muls + a small add tree), and the very
  last k-slice is DMA'd and combined per 512-col bank, so the critical path
  after the final input DMA is a few ~0.35us ops + one small out DMA --
  NOT a cold-PE matmul burst (the cost model's PE pstate ramp makes
  dribbled matmuls ~4x slower, so the tail must avoid PE).

The host pre-pads data in h and w; the kernel loads 5 row-shifted copies of
the padded data (row shifts cannot be done on-chip: engine/DMA SBUF access
patterns may only start at partitions 0/32/64/96).
"""

import numpy as np
import ml_dtypes

B, C, H, W = 4, 64, 128, 128
K, PAD, KK = 5, 2, 25
NCORES = 8
CPC = C // NCORES            # channels per core = 8
NIMG = B * CPC               # channel-images per core = 32
GRP = 16                     # images per compute group
NG = NIMG // GRP             # groups = 2
WP = W + 2 * PAD             # 132
HP = H + 2 * PAD             # 132
GW = GRP * W                 # 2048 free columns per group
CHUNK = 512                  # PSUM bank = 512 f32 per partition
NCH = GW // CHUNK            # 4 bank chunks per group

BF16 = ml_dtypes.bfloat16

_CACHE = {}


def _build_module(debug=False):
    import concourse.tile as tile
    from concourse import bacc, bass, mybir

    f32 = mybir.dt.float32
    bf16 = mybir.dt.bfloat16
    nc = bacc.Bacc(
        "TRN2", target_bir_lowering=False, debug=debug, num_devices=NCORES
    )
    # core k-slices stored j-major: slice k' = j*K + i holds tap (i, j)
    core_d = nc.dram_tensor(
        "core", [KK, H, NIMG * W], bf16, kind="ExternalInput"
    ).ap()
    data_d = nc.dram_tensor(
        "data", [HP, NIMG * WP], bf16, kind="ExternalInput"
    ).ap()
    ident_d = nc.dram_tensor("ident", [128, 128], bf16, kind="ExternalInput").ap()
    out_d = nc.dram_tensor("out", [H, NIMG * W], bf16, kind="ExternalOutput").ap()

    with tile.TileContext(nc) as tc:
        with (
            tc.tile_pool(name="constp", bufs=1) as k_pool,
            tc.tile_pool(name="shifts", bufs=NG) as sh_pool,
            tc.tile_pool(name="corep", bufs=2) as c_pool,
            tc.tile_pool(name="corej4", bufs=4) as c4_pool,
            tc.tile_pool(name="corebk", bufs=4) as cb_pool,
            tc.tile_pool(name="prodp", bufs=2) as p_pool,
            tc.tile_pool(name="prodj4", bufs=4) as p4_pool,
            tc.tile_pool(name="prodbk", bufs=4) as pb_pool,
            tc.tile_pool(name="treep", bufs=1) as t_pool,
            tc.tile_pool(name="outp", bufs=2) as o_pool,
            tc.tile_pool(
                name="psump", bufs=NG, space=bass.MemorySpace.PSUM
            ) as ps_pool,
        ):
            ident = k_pool.tile([128, 128], bf16, tag="ident")
            nc.scalar.dma_start(ident[:], ident_d[:, :])
            # all shift copies up front so nothing on the scalar ring blocks
            # a later group's data loads behind a PSUM-drain dependency
            shs = []
            for g in range(NG):
                gwp = slice(g * GRP * WP, (g + 1) * GRP * WP)
                sh = sh_pool.tile([H, K * GRP * WP], bf16, tag="sh", name=f"sh{g}")
                for i in range(K):
                    # contiguous 2D APs on both sides: fewest DMA descriptors
                    # (3D APs overflow walrus's 16-bit IO-DGE
                    # semaphore_wait_value field)
                    nc.scalar.dma_start(
                        sh[:, i * GRP * WP : (i + 1) * GRP * WP],
                        data_d[i : i + H, gwp],
                    )
                shs.append(sh.rearrange("p (s b w) -> p s b w", s=K, b=GRP))
            for g in range(NG):
                gw = slice(g * GW, (g + 1) * GW)
                sh4 = shs[g]
                ps = ps_pool.tile([H, GW], f32, tag="ps", name=f"ps{g}")
                # ---- taps j=0..3: DVE products, PE identity-matmul accum ----
                for j in range(K - 1):
                    ct = c_pool.tile([H, K * GW], bf16, tag="ct")
                    for i in range(K):
                        nc.sync.dma_start(
                            ct[:, i * GW : (i + 1) * GW],
                            core_d[j * K + i, :, gw],
                        )
                    pt = p_pool.tile([H, K * GW], bf16, tag="pt")
                    ct4 = ct.rearrange("p (s b w) -> p s b w", s=K, b=GRP)
                    pt4 = pt.rearrange("p (s b w) -> p s b w", s=K, b=GRP)
                    nc.vector.tensor_mul(pt4, ct4, sh4[:, :, :, j : j + W])
                    for i in range(K):
                        for c in range(NCH):
                            off = i * GW + c * CHUNK
                            nc.tensor.matmul(
                                ps[:, c * CHUNK : (c + 1) * CHUNK],
                                ident[:],
                                pt[:, off : off + CHUNK],
                                start=(j == 0 and i == 0),
                                stop=(j == K - 2 and i == K - 1),
                            )
                # drain the 20-tap partial sums while j=4 is still loading
                pss = o_pool.tile([H, GW], bf16, tag="pss")
                nc.scalar.copy(pss[:], ps[:])
                # ---- taps j=4: DVE-only accumulation (tail stays off PE) ----
                j = K - 1
                prods = []
                for i in range(K - 1):
                    cti = c4_pool.tile([H, GW], bf16, tag="ct4", name=f"ct4_{g}_{i}")
                    nc.sync.dma_start(cti[:], core_d[j * K + i, :, gw])
                    pti = p4_pool.tile([H, GW], bf16, tag="pt4", name=f"pt4_{g}_{i}")
                    p3 = pti.rearrange("p (b w) -> p b w", b=GRP)
                    nc.vector.tensor_mul(
                        p3,
                        cti.rearrange("p (b w) -> p b w", b=GRP),
                        sh4[:, i, :, j : j + W],
                    )
                    prods.append(pti)
                ta = t_pool.tile([H, GW], bf16, tag="ta", name=f"ta{g}")
                tb = t_pool.tile([H, GW], bf16, tag="tb", name=f"tb{g}")
                nc.vector.tensor_add(ta[:], prods[0][:], prods[1][:])
                nc.vector.tensor_add(tb[:], prods[2][:], prods[3][:])
                nc.vector.tensor_add(ta[:], ta[:], tb[:])  # taps j4,i0..3
                # last k-slice per 512-col bank: minimal post-DMA chain
                ot = o_pool.tile([H, GW], bf16, tag="ot")
                ta3 = ta.rearrange("p (b w) -> p b w", b=GRP)
                ot3 = ot.rearrange("p (b w) -> p b w", b=GRP)
                pss3 = pss.rearrange("p (b w) -> p b w", b=GRP)
                bpc = CHUNK // W  # images per 512-col bank = 4
                for c in range(NCH):
                    bs = slice(c * bpc, (c + 1) * bpc)
                    gwc = slice(g * GW + c * CHUNK, g * GW + (c + 1) * CHUNK)
                    cbc = cb_pool.tile(
                        [H, CHUNK], bf16, tag="cb", name=f"cb{g}_{c}"
                    )
                    nc.sync.dma_start(cbc[:], core_d[j * K + K - 1, :, gwc])
                    pbc = pb_pool.tile(
                        [H, CHUNK], bf16, tag="pb", name=f"pb{g}_{c}"
                    )
                    pbc3 = pbc.rearrange("p (b w) -> p b w", b=bpc)
                    nc.vector.tensor_mul(
                        pbc3,
                        cbc.rearrange("p (b w) -> p b w", b=bpc),
                        sh4[:, K - 1, bs, j : j + W],
                    )
                    nc.vector.tensor_add(pbc3, pbc3, ta3[:, bs])
                    nc.vector.tensor_add(ot3[:, bs], pbc3, pss3[:, bs])
                    nc.scalar.dma_start(out_d[:, gwc], ot[:, c * CHUNK : (c + 1) * CHUNK])

    nc.compile()
    return nc


def get_nc(debug=False):
    key = ("nc", debug)
    if key not in _CACHE:
        _CACHE[key] = _build_module(debug=debug)
    return _CACHE[key]


def prep_inputs(data, core):
    """Full inputs -> list of per-core input dicts (host-side shard + pad)."""
    data = np.asarray(data, dtype=np.float32)
    core = np.asarray(core, dtype=np.float32)
    # [b, i, j, c, h, w] -> [j, i, h, b, c, w]: k-slices j-major on device
    core_t = np.ascontiguousarray(
        core.reshape(B, K, K, C, H, W).transpose(2, 1, 4, 0, 3, 5).astype(BF16)
    )
    dp = np.zeros((HP, B, C, WP), BF16)
    dp[PAD : PAD + H, :, :, PAD : PAD + W] = data.transpose(2, 0, 1, 3)
    ident = np.ascontiguousarray(np.eye(128, dtype=BF16))
    in_maps = []
    for r in range(NCORES):
        cs = slice(r * CPC, (r + 1) * CPC)
        core_r = np.ascontiguousarray(core_t[:, :, :, :, cs, :]).reshape(
            KK, H, NIMG * W
        )
        data_r = np.ascontiguousarray(dp[:, :, cs, :]).reshape(HP, NIMG * WP)
        in_maps.append({"core": core_r, "data": data_r, "ident": ident})
    return in_maps


def assemble(per_core_outs):
    """Per-core 'out' arrays [H, NIMG*W] (bf16) -> full [B, C, H, W] f32."""
    out = np.empty((B, C, H, W), np.float32)
    for r, o in enumerate(per_core_outs):
        cs = slice(r * CPC, (r + 1) * CPC)
        out[:, cs] = (
            np.asarray(o).astype(np.float32)
            .reshape(H, B, CPC, W)
            .transpose(1, 2, 0, 3)
        )
    return out


def run_spmd(in_maps, trace=False, trace_cores=None):
    from concourse.bass_utils import run_bass_kernel_spmd

    return run_bass_kernel_spmd(
        get_nc(),
        in_maps,
        list(range(NCORES)),
        trace=trace,
        trace_cores=trace_cores,
    )


def kernel(data, core):
    res = run_spmd(prep_inputs(data, core))
    return assemble([res.results[r]["out"] for r in range(NCORES)])


# revision 6
# speedup vs baseline: 1.8118x; 1.0069x over previous
"""Per-pixel dynamic 5x5 conv (KernelConv) on 8 Trainium2 NeuronCores.

out[b,c,h,w] = sum_{i,j} core[b,(i*5+j)*C+c,h,w] * pad(data)[b,c,h+i,w+j]

Sharding: channel groups of 8 per core (x 4 batches = 32 channel-images/core).
Layout on chip: partitions = h (128), free dim = image-blocks x w.

All HBM traffic is bf16 (host converts): halves the DMA bytes vs f32, and the
2e-2 rel-err budget dwarfs bf16 rounding (~0.5% worst case here).

Compute split so no engine exceeds the ~91us bf16 DMA roofline:
- DVE does the 25 products, batched 5 taps per instruction for j<4 (all-bf16
  operands hit the DVE 2x perf mode; a mixed f32 output would forfeit it).
  Host stores core k-slices j-major (k' = j*5+i) so one instruction pairs
  core chunk j with all 5 row-shift copies via a 3-free-dim AP.
- Taps j=0..3 accumulate on the otherwise-idle TensorEngine as identity
  matmuls into PSUM (f32, exact); ACT drains PSUM -> bf16 SBUF as soon as
  the 20th tap lands.
- Taps j=4 accumulate on DVE (per-k muls + a small add tree), and the very
  last k-slice is DMA'd and combined per 512-col bank, so the critical path
  after the final input DMA is a few ~0.35us ops + one small out DMA --
  NOT a cold-PE matmul burst (the cost model's PE pstate ramp makes
  dribbled matmuls ~4x slower, so the tail must avoid PE).

The host pre-pads data in h and w; the kernel loads 5 row-shifted copies of
the padded data (row shifts cannot be done on-chip: engine/DMA SBUF access
patterns may only start at partitions 0/32/64/96).
"""

import numpy as np
import ml_dtypes

B, C, H, W = 4, 64, 128, 128
K, PAD, KK = 5, 2, 25
NCORES = 8
CPC = C // NCORES            # channels per core = 8
NIMG = B * CPC               # channel-images per core = 32
GRP = 16                     # images per compute group
NG = NIMG // GRP             # groups = 2
WP = W + 2 * PAD             # 132
HP = H + 2 * PAD             # 132
GW = GRP * W                 # 2048 free columns per group
CHUNK = 512                  # PSUM bank = 512 f32 per partition
NCH = GW // CHUNK            # 4 bank chunks per group

BF16 = ml_dtypes.bfloat16

_CACHE = {}


def _build_module(debug=False):
    import concourse.tile as tile
    from concourse import bacc, bass, mybir

    f32 = mybir.dt.float32
    bf16 = mybir.dt.bfloat16
    nc = bacc.Bacc(
        "TRN2", target_bir_lowering=False, debug=debug, num_devices=NCORES
    )
    # core k-slices stored j-major: slice k' = j*K + i holds tap (i, j)
    core_d = nc.dram_tensor(
        "core", [KK, H, NIMG * W], bf16, kind="ExternalInput"
    ).ap()
    data_d = nc.dram_tensor(
        "data", [HP, NIMG * WP], bf16, kind="ExternalInput"
    ).ap()
    ident_d = nc.dram_tensor("ident", [128, 128], bf16, kind="ExternalInput").ap()
    out_d = nc.dram_tensor("out", [H, NIMG * W], bf16, kind="ExternalOutput").ap()

    with tile.TileContext(nc) as tc:
        with (
            tc.tile_pool(name="constp", bufs=1) as k_pool,
            tc.tile_pool(name="shifts", bufs=NG) as sh_pool,
            tc.tile_pool(name="corep", bufs=3) as c_pool,
            tc.tile_pool(name="corej4", bufs=4) as c4_pool,
            tc.tile_pool(name="corebk", bufs=4) as cb_pool,
            tc.tile_pool(name="prodp", bufs=2) as p_pool,
            tc.tile_pool(name="prodj4", bufs=4) as p4_pool,
            tc.tile_pool(name="prodbk", bufs=4) as pb_pool,
            tc.tile_pool(name="outp", bufs=2) as o_pool,
            tc.tile_pool(
                name="psump", bufs=NG, space=bass.MemorySpace.PSUM
            ) as ps_pool,
        ):
            ident = k_pool.tile([128, 128], bf16, tag="ident")
            nc.scalar.dma_start(ident[:], ident_d[:, :])
            # all shift copies up front so nothing on the scalar ring blocks
            # a later group's data loads behind a PSUM-drain dependency
            shs = []
            for g in range(NG):
                gwp = slice(g * GRP * WP, (g + 1) * GRP * WP)
                sh = sh_pool.tile([H, K * GRP * WP], bf16, tag="sh", name=f"sh{g}")
                for i in range(K):
                    # contiguous 2D APs on both sides: fewest DMA descriptors
                    # (3D APs overflow walrus's 16-bit IO-DGE
                    # semaphore_wait_value field)
                    nc.scalar.dma_start(
                        sh[:, i * GRP * WP : (i + 1) * GRP * WP],
                        data_d[i : i + H, gwp],
                    )
                shs.append(sh.rearrange("p (s b w) -> p s b w", s=K, b=GRP))
            for g in range(NG):
                gw = slice(g * GW, (g + 1) * GW)
                sh4 = shs[g]
                ps = ps_pool.tile([H, GW], f32, tag="ps", name=f"ps{g}")
                # ---- taps j=0..2: batched DVE products, PE 20-matmul bursts
                for j in range(K - 2):
                    ct = c_pool.tile([H, K * GW], bf16, tag="ct")
                    for i in range(K):
                        nc.sync.dma_start(
                            ct[:, i * GW : (i + 1) * GW],
                            core_d[j * K + i, :, gw],
                        )
                    pt = p_pool.tile([H, K * GW], bf16, tag="pt")
                    ct4 = ct.rearrange("p (s b w) -> p s b w", s=K, b=GRP)
                    pt4 = pt.rearrange("p (s b w) -> p s b w", s=K, b=GRP)
                    nc.vector.tensor_mul(pt4, ct4, sh4[:, :, :, j : j + W])
                    for i in range(K):
                        for c in range(NCH):
                            off = i * GW + c * CHUNK
                            nc.tensor.matmul(
                                ps[:, c * CHUNK : (c + 1) * CHUNK],
                                ident[:],
                                pt[:, off : off + CHUNK],
                                start=(j == 0 and i == 0),
                                stop=False,
                            )
                # ---- taps j=3, j=4 (i<4): per-k muls + 4-matmul bites so the
                # tail never waits on a 5-wide chunk; the cost model keeps PE
                # at full speed across these short gaps (verified).
                for j, i in [(K - 2, i) for i in range(K)] + [
                    (K - 1, i) for i in range(K - 1)
                ]:
                    cti = c4_pool.tile(
                        [H, GW], bf16, tag="ct4", name=f"ct4_{g}_{j}_{i}"
                    )
                    nc.sync.dma_start(cti[:], core_d[j * K + i, :, gw])
                    pti = p4_pool.tile(
                        [H, GW], bf16, tag="pt4", name=f"pt4_{g}_{j}_{i}"
                    )
                    nc.vector.tensor_mul(
                        pti.rearrange("p (b w) -> p b w", b=GRP),
                        cti.rearrange("p (b w) -> p b w", b=GRP),
                        sh4[:, i, :, j : j + W],
                    )
                    for c in range(NCH):
                        nc.tensor.matmul(
                            ps[:, c * CHUNK : (c + 1) * CHUNK],
                            ident[:],
                            pti[:, c * CHUNK : (c + 1) * CHUNK],
                            start=False,
                            stop=False,
                        )
                # ---- last k-slice (j=4, i=4) per 512-col bank: stop each
                # bank's PSUM group, ACT-drain it to bf16, DMA it out.
                ot = o_pool.tile([H, GW], bf16, tag="ot")
                j = K - 1
                bpc = CHUNK // W  # images per 512-col bank = 4
                for c in range(NCH):
                    cs = slice(c * CHUNK, (c + 1) * CHUNK)
                    bs = slice(c * bpc, (c + 1) * bpc)
                    gwc = slice(g * GW + c * CHUNK, g * GW + (c + 1) * CHUNK)
                    cbc = cb_pool.tile(
                        [H, CHUNK], bf16, tag="cb", name=f"cb{g}_{c}"
                    )
                    nc.sync.dma_start(cbc[:], core_d[j * K + K - 1, :, gwc])
                    pbc = pb_pool.tile(
                        [H, CHUNK], bf16, tag="pb", name=f"pb{g}_{c}"
                    )
                    nc.vector.tensor_mul(
                        pbc.rearrange("p (b w) -> p b w", b=bpc),
                        cbc.rearrange("p (b w) -> p b w", b=bpc),
                        sh4[:, K - 1, bs, j : j + W],
                    )
                    nc.tensor.matmul(
                        ps[:, cs], ident[:], pbc[:], start=False, stop=True
                    )
                    nc.scalar.copy(ot[:, cs], ps[:, cs])
                    nc.scalar.dma_start(out_d[:, gwc], ot[:, cs])

    nc.compile()
    return nc


def get_nc(debug=False):
    key = ("nc", debug)
    if key not in _CACHE:
        _CACHE[key] = _build_module(debug=debug)
    return _CACHE[key]


def prep_inputs(data, core):
    """Full inputs -> list of per-core input dicts (host-side shard + pad)."""
    data = np.asarray(data, dtype=np.float32)
    core = np.asarray(core, dtype=np.float32)
    # [b, i, j, c, h, w] -> [j, i, h, b, c, w]: k-slices j-major on device
    core_t = np.ascontiguousarray(
        core.reshape(B, K, K, C, H, W).transpose(2, 1, 4, 0, 3, 5).astype(BF16)
    )
    dp = np.zeros((HP, B, C, WP), BF16)
    dp[PAD : PAD + H, :, :, PAD : PAD + W] = data.transpose(2, 0, 1, 3)
    ident = np.ascontiguousarray(np.eye(128, dtype=BF16))
    in_maps = []
    for r in range(NCORES):
        cs = slice(r * CPC, (r + 1) * CPC)
        core_r = np.ascontiguousarray(core_t[:, :, :, :, cs, :]).reshape(
            KK, H, NIMG * W
        )
        data_r = np.ascontiguousarray(dp[:, :, cs, :]).reshape(HP, NIMG * WP)
        in_maps.append({"core": core_r, "data": data_r, "ident": ident})
    return in_maps


def assemble(per_core_outs):
    """Per-core 'out' arrays [H, NIMG*W] (bf16) -> full [B, C, H, W] f32."""
    out = np.empty((B, C, H, W), np.float32)
    for r, o in enumerate(per_core_outs):
        cs = slice(r * CPC, (r + 1) * CPC)
        out[:, cs] = (
            np.asarray(o).astype(np.float32)
            .reshape(H, B, CPC, W)
            .transpose(1, 2, 0, 3)
        )
    return out


def run_spmd(in_maps, trace=False, trace_cores=None):
    from concourse.bass_utils import run_bass_kernel_spmd

    return run_bass_kernel_spmd(
        get_nc(),
        in_maps,
        list(range(NCORES)),
        trace=trace,
        trace_cores=trace_cores,
    )


def kernel(data, core):
    res = run_spmd(prep_inputs(data, core))
    return assemble([res.results[r]["out"] for r in range(NCORES)])


# revision 20
# speedup vs baseline: 2.0361x; 1.1238x over previous
"""Per-pixel dynamic 5x5 conv (KernelConv) on 8 Trainium2 NeuronCores.

out[b,c,h,w] = sum_{i,j} core[b,(i*5+j)*C+c,h,w] * pad(data)[b,c,h+i,w+j]

Sharding: channel groups of 8 per core (x 4 batches = 32 channel-images/core).
Layout on chip: partitions = h (128), free dim = image-blocks x w.

All HBM traffic is bf16 (host converts): halves the DMA bytes vs f32, and the
2e-2 rel-err budget dwarfs bf16 rounding (~0.5% worst case here).

Compute split so no engine exceeds the ~91us bf16 DMA roofline:
- DVE does the 25 products, batched 5 taps per instruction for j<4 (all-bf16
  operands hit the DVE 2x perf mode; a mixed f32 output would forfeit it).
  Host stores core k-slices j-major (k' = j*5+i) so one instruction pairs
  core chunk j with all 5 row-shift copies via a 3-free-dim AP.
- Taps j=0..3 accumulate on the otherwise-idle TensorEngine as identity
  matmuls into PSUM (f32, exact); ACT drains PSUM -> bf16 SBUF as soon as
  the 20th tap lands.
- Taps j=4 accumulate on DVE (per-k muls + a small add tree), and the very
  last k-slice is DMA'd and combined per 512-col bank, so the critical path
  after the final input DMA is a few ~0.35us ops + one small out DMA --
  NOT a cold-PE matmul burst (the cost model's PE pstate ramp makes
  dribbled matmuls ~4x slower, so the tail must avoid PE).

The host pre-pads data in h and w; the kernel loads 5 row-shifted copies of
the padded data (row shifts cannot be done on-chip: engine/DMA SBUF access
patterns may only start at partitions 0/32/64/96).
"""

import numpy as np
import ml_dtypes

B, C, H, W = 4, 64, 128, 128
K, PAD, KK = 5, 2, 25
NCORES = 8
CPC = C // NCORES            # channels per core = 8
NIMG = B * CPC               # channel-images per core = 32
GRPS = (24, 8)               # images per compute group: last kept small so the
                             # final drain/out chain covers few PSUM banks, but
                             # big enough that its slice DMAs (728ns) outpace
                             # the 565ns per-DMA ring issue rate
NG = len(GRPS)
WP = W + 2 * PAD             # 132
HP = H + 2 * PAD             # 132
CHUNK = 512                  # PSUM bank = 512 f32 per partition

BF16 = ml_dtypes.bfloat16

_CACHE = {}


def _build_module(debug=False):
    import concourse.tile as tile
    from concourse import bacc, bass, mybir

    f32 = mybir.dt.float32
    bf16 = mybir.dt.bfloat16
    nc = bacc.Bacc(
        "TRN2", target_bir_lowering=False, debug=debug, num_devices=NCORES
    )
    # core k-slices stored j-major: slice k' = j*K + i holds tap (i, j)
    core_d = nc.dram_tensor(
        "core", [KK, H, NIMG * W], bf16, kind="ExternalInput"
    ).ap()
    data_d = nc.dram_tensor(
        "data", [HP, NIMG * WP], bf16, kind="ExternalInput"
    ).ap()
    ident_d = nc.dram_tensor("ident", [128, 128], bf16, kind="ExternalInput").ap()
    out_d = nc.dram_tensor("out", [H, NIMG * W], bf16, kind="ExternalOutput").ap()

    with tile.TileContext(nc) as tc:
        with (
            tc.tile_pool(name="constp", bufs=1) as k_pool,
            tc.tile_pool(name="shifts", bufs=1) as sh_pool,
            tc.tile_pool(name="corep", bufs=8) as c_pool,
            tc.tile_pool(name="prodp", bufs=6) as p_pool,
            tc.tile_pool(name="outp", bufs=8) as o_pool,
            tc.tile_pool(
                name="psump", bufs=8, space=bass.MemorySpace.PSUM
            ) as ps_pool,
        ):
            ident = k_pool.tile([128, 128], bf16, tag="ident")
            nc.scalar.dma_start(ident[:], ident_d[:, :])
            # all shift copies up front so nothing on the scalar ring blocks
            # a later group's data loads behind a PSUM-drain dependency
            shs = []
            for g, grp in enumerate(GRPS):
                g0 = sum(GRPS[:g])
                gwp = slice(g0 * WP, (g0 + grp) * WP)
                sh = sh_pool.tile(
                    [H, K * grp * WP], bf16, tag=f"sh{g}", name=f"sh{g}"
                )
                for i in range(K):
                    # contiguous 2D APs on both sides: fewest DMA descriptors
                    # (3D APs overflow walrus's 16-bit IO-DGE
                    # semaphore_wait_value field)
                    nc.scalar.dma_start(
                        sh[:, i * grp * WP : (i + 1) * grp * WP],
                        data_d[i : i + H, gwp],
                    )
                shs.append(sh.rearrange("p (s b w) -> p s b w", s=K, b=grp))
            outs_pending = []
            for g, grp in enumerate(GRPS):
                g0 = sum(GRPS[:g])
                gw_cols = grp * W
                nch = gw_cols // CHUNK
                gw = slice(g0 * W, g0 * W + gw_cols)
                sh4 = shs[g]
                last_g = g == NG - 1
                # one PSUM tile per 512-col bank: a monolithic tile creates
                # tile-level WARs between one bank's ACT drain and the next
                # bank's stop-matmul, serializing the tail ~2us per bank
                pss = [
                    ps_pool.tile([H, CHUNK], f32, tag="ps", name=f"ps{g}_{c}")
                    for c in range(nch)
                ]
                # all taps per-k: DVE mul + nch-matmul PE bite each. DVE
                # outruns the slice DMA so nothing backs up, and the cost
                # model keeps PE at full speed across the short gaps between
                # bites (verified by probe).
                for j in range(K):
                    for i in range(K):
                        cti = c_pool.tile(
                            [H, gw_cols], bf16, tag=f"ct{g}",
                            name=f"ct_{g}_{j}_{i}",
                        )
                        nc.sync.dma_start(cti[:], core_d[j * K + i, :, gw])
                        pti = p_pool.tile(
                            [H, gw_cols], bf16, tag=f"pt{g}",
                            name=f"pt_{g}_{j}_{i}",
                        )
                        nc.vector.tensor_mul(
                            pti.rearrange("p (b w) -> p b w", b=grp),
                            cti.rearrange("p (b w) -> p b w", b=grp),
                            sh4[:, i, :, j : j + W],
                        )
                        stop = j == K - 1 and i == K - 1
                        for c in range(nch):
                            nc.tensor.matmul(
                                pss[c][:],
                                ident[:],
                                pti[:, c * CHUNK : (c + 1) * CHUNK],
                                start=(j == 0 and i == 0),
                                stop=stop,
                            )
                # per-bank ACT drain to bf16, then out DMAs. ALL out issues
                # sit on the sync ring AFTER the last group's core loads in
                # program order: issued earlier, the hidden groups' out
                # transfers would interleave into the input stream and push
                # the last input (and the whole tail chain) ~1.5us later,
                # while the post-input DMA window sat idle.
                for c in range(nch):
                    gwc = slice(
                        g0 * W + c * CHUNK, g0 * W + (c + 1) * CHUNK
                    )
                    otc = o_pool.tile(
                        [H, CHUNK], bf16, tag="ot", name=f"ot{g}_{c}"
                    )
                    nc.scalar.copy(otc[:], pss[c][:])
                    outs_pending.append((gwc, otc))
                if last_g:
                    for gwc, otc in outs_pending:
                        nc.sync.dma_start(out_d[:, gwc], otc[:])

    nc.compile()
    return nc


def get_nc(debug=False):
    key = ("nc", debug)
    if key not in _CACHE:
        _CACHE[key] = _build_module(debug=debug)
    return _CACHE[key]


def prep_inputs(data, core):
    """Full inputs -> list of per-core input dicts (host-side shard + pad)."""
    data = np.asarray(data, dtype=np.float32)
    core = np.asarray(core, dtype=np.float32)
    # [b, i, j, c, h, w] -> [j, i, h, b, c, w]: k-slices j-major on device
    core_t = np.ascontiguousarray(
        core.reshape(B, K, K, C, H, W).transpose(2, 1, 4, 0, 3, 5).astype(BF16)
    )
    dp = np.zeros((HP, B, C, WP), BF16)
    dp[PAD : PAD + H, :, :, PAD : PAD + W] = data.transpose(2, 0, 1, 3)
    ident = np.ascontiguousarray(np.eye(128, dtype=BF16))
    in_maps = []
    for r in range(NCORES):
        cs = slice(r * CPC, (r + 1) * CPC)
        core_r = np.ascontiguousarray(core_t[:, :, :, :, cs, :]).reshape(
            KK, H, NIMG * W
        )
        data_r = np.ascontiguousarray(dp[:, :, cs, :]).reshape(HP, NIMG * WP)
        in_maps.append({"core": core_r, "data": data_r, "ident": ident})
    return in_maps


def assemble(per_core_outs):
    """Per-core 'out' arrays [H, NIMG*W] (bf16) -> full [B, C, H, W] f32."""
    out = np.empty((B, C, H, W), np.float32)
    for r, o in enumerate(per_core_outs):
        cs = slice(r * CPC, (r + 1) * CPC)
        out[:, cs] = (
            np.asarray(o).astype(np.float32)
            .reshape(H, B, CPC, W)
            .transpose(1, 2, 0, 3)
        )
    return out


def run_spmd(in_maps, trace=False, trace_cores=None):
    from concourse.bass_utils import run_bass_kernel_spmd

    return run_bass_kernel_spmd(
        get_nc(),
        in_maps,
        list(range(NCORES)),
        trace=trace,
        trace_cores=trace_cores,
    )


def kernel(data, core):
    res = run_spmd(prep_inputs(data, core))
    return assemble([res.results[r]["out"] for r in range(NCORES)])


# revision 23
# speedup vs baseline: 2.1506x; 1.0562x over previous
"""Per-pixel dynamic 5x5 conv (KernelConv) on 8 Trainium2 NeuronCores.

out[b,c,h,w] = sum_{i,j} core[b,(i*5+j)*C+c,h,w] * pad(data)[b,c,h+i,w+j]

Sharding: channel groups of 8 per core (x 4 batches = 32 channel-images/core).
Layout on chip: partitions = h (128), free dim = image-blocks x w.

All HBM traffic is bf16 (host converts): halves the DMA bytes vs f32, and the
2e-2 rel-err budget dwarfs bf16 rounding (~0.5% worst case here).

The DMA roofline (360 GB/s shared across all queues in the cost model) is the
binding constraint, so the kernel minimizes bytes moved:
- core: 26.2 MB/core, read once as 100 per-k group slices.
- data: loaded ONCE (padded rows 0..127 full-width + a 4-row bottom tile).
  The other four row-shifted copies the taps need are built ON CHIP by the
  TensorEngine: shifted-identity matmuls (S_i = eye shifted by i, plus a
  4-row fixup from the bottom tile) into PSUM, ACT-copied back to bf16 SBUF
  512-col chunks at a time. This replaces ~4.3 MB of duplicate HBM reads.
  (Row shifts cannot be done with plain engine copies: SBUF access patterns
  may only start at partitions 0/32/64/96; only PE matmul or DMA cross
  partitions, and in the cost model DMA bandwidth is the bottleneck.)
- out: written as bf16, per 512-col PSUM bank.

Compute structure (all-bf16 operands keep the DVE 2x perf mode; a mixed f32
output would forfeit it):
- DVE does only the 25 per-k products per group ([128,1024] each).
- The 24 adds per pixel run on the TensorEngine as identity matmuls
  accumulating into PSUM (f32, exact): 2 bank-matmuls per product. Taps run
  i-major so plane i+1 is PE-built (interleaved into round i's tap stream)
  just before round i+1 consumes it.
- ACT drains each finished PSUM bank to bf16; all out DMAs are issued on the
  sync ring after the last group's loads so their transfers fill the
  post-input DMA window instead of delaying the last input.
- 4 groups of 8 images: PSUM = 4 accum banks (reused across groups) + 4
  shift-chunk banks; the small last group keeps the final
  mul->matmul->drain->out chain short, and 728ns slice DMAs still outpace
  the 565ns per-DMA ring issue rate.
"""

import numpy as np
import ml_dtypes

B, C, H, W = 4, 64, 128, 128
K, PAD, KK = 5, 2, 25
NCORES = 8
CPC = C // NCORES            # channels per core = 8
NIMG = B * CPC               # channel-images per core = 32
GRPS = (16, 8, 8)            # images per compute group: first is wide so its
                             # 7.3us tap rounds outpace the 5.4us ACT-bound
                             # plane builds; last is small for a short tail
NG = len(GRPS)
WP = W + 2 * PAD             # 132
HP = H + 2 * PAD             # 132
CHUNK = 512                  # PSUM bank = 512 f32 per partition
FW = NIMG * WP               # full padded width = 4224

BF16 = ml_dtypes.bfloat16

_CACHE = {}


def _shift_chunks():
    out = []
    off = 0
    while off < FW:
        n = min(CHUNK, FW - off)
        out.append((off, n))
        off += n
    return out


def _build_module(debug=False):
    import concourse.tile as tile
    from concourse import bacc, bass, mybir

    f32 = mybir.dt.float32
    bf16 = mybir.dt.bfloat16
    nc = bacc.Bacc(
        "TRN2", target_bir_lowering=False, debug=debug, num_devices=NCORES
    )
    core_d = nc.dram_tensor(
        "core", [KK, H, NIMG * W], bf16, kind="ExternalInput"
    ).ap()
    data_d = nc.dram_tensor(
        "data", [HP, FW], bf16, kind="ExternalInput"
    ).ap()
    ident_d = nc.dram_tensor("ident", [128, 128], bf16, kind="ExternalInput").ap()
    smat_d = nc.dram_tensor(
        "smat", [128, (K - 1) * 128], bf16, kind="ExternalInput"
    ).ap()
    fmat_d = nc.dram_tensor(
        "fmat", [2 * PAD, (K - 1) * 128], bf16, kind="ExternalInput"
    ).ap()
    out_d = nc.dram_tensor("out", [H, NIMG * W], bf16, kind="ExternalOutput").ap()

    chunks = _shift_chunks()

    with tile.TileContext(nc) as tc:
        with (
            tc.tile_pool(name="constp", bufs=1) as k_pool,
            tc.tile_pool(name="shifts", bufs=1) as sh_pool,
            tc.tile_pool(name="corep", bufs=8) as ca_pool,
            tc.tile_pool(name="corepb", bufs=12) as cb_pool,
            tc.tile_pool(name="prodp", bufs=6) as pa_pool,
            tc.tile_pool(name="prodpb", bufs=8) as pb_pool,
            tc.tile_pool(name="outp", bufs=NG * 2) as o_pool,
            tc.tile_pool(
                name="psump", bufs=4, space=bass.MemorySpace.PSUM
            ) as ps_pool,
            tc.tile_pool(
                name="pshp", bufs=4, space=bass.MemorySpace.PSUM
            ) as psh_pool,
        ):
            ident = k_pool.tile([128, 128], bf16, tag="ident")
            nc.scalar.dma_start(ident[:], ident_d[:, :])
            smat = k_pool.tile([128, (K - 1) * 128], bf16, tag="smat")
            nc.scalar.dma_start(smat[:], smat_d[:, :])
            fmat = k_pool.tile([2 * PAD, (K - 1) * 128], bf16, tag="fmat")
            nc.scalar.dma_start(fmat[:], fmat_d[:, :])
            dpb = k_pool.tile([2 * PAD, FW], bf16, tag="dpb")
            nc.scalar.dma_start(dpb[:], data_d[H:HP, :])
            # full-width shift planes, one tile per row shift i so a tap's
            # product waits only on its own plane's writers
            shp = []
            for i in range(K):
                t = sh_pool.tile([H, FW], bf16, tag=f"shp{i}", name=f"shp{i}")
                shp.append(t)
            nc.scalar.dma_start(shp[0][:], data_d[0:H, :])

            def emit_shift_chunk(i, ci):
                off, n = chunks[ci]
                psh = psh_pool.tile(
                    [H, CHUNK], f32, tag="psh", name=f"psh{i}_{ci}"
                )
                nc.tensor.matmul(
                    psh[:, :n],
                    smat[:, (i - 1) * 128 : i * 128],
                    shp[0][:, off : off + n],
                    start=True,
                    stop=False,
                )
                nc.tensor.matmul(
                    psh[:, :n],
                    fmat[:, (i - 1) * 128 : i * 128],
                    dpb[:, off : off + n],
                    start=False,
                    stop=True,
                )
                nc.scalar.copy(shp[i][:, off : off + n], psh[:, :n])

            outs_pending = []
            for g, grp in enumerate(GRPS):
                g0 = sum(GRPS[:g])
                gw_cols = grp * W
                nch = gw_cols // CHUNK
                gw = slice(g0 * W, g0 * W + gw_cols)
                last_g = g == NG - 1
                pss = [
                    ps_pool.tile([H, CHUNK], f32, tag="ps", name=f"ps{g}_{c}")
                    for c in range(nch)
                ]
                for i in range(K):
                    for j in range(K):
                        cti = (ca_pool if grp == 16 else cb_pool).tile(
                            [H, gw_cols], bf16, tag=f"ct{grp}",
                            name=f"ct_{g}_{i}_{j}",
                        )
                        nc.sync.dma_start(cti[:], core_d[i * K + j, :, gw])
                        pti = (pa_pool if grp == 16 else pb_pool).tile(
                            [H, gw_cols], bf16, tag=f"pt{grp}",
                            name=f"pt_{g}_{i}_{j}",
                        )
                        shv = shp[i].rearrange("p (b w) -> p b w", b=NIMG)
                        nc.vector.tensor_mul(
                            pti.rearrange("p (b w) -> p b w", b=grp),
                            cti.rearrange("p (b w) -> p b w", b=grp),
                            shv[:, g0 : g0 + grp, j : j + W],
                        )
                        stop = i == K - 1 and j == K - 1
                        for c in range(nch):
                            nc.tensor.matmul(
                                pss[c][:],
                                ident[:],
                                pti[:, c * CHUNK : (c + 1) * CHUNK],
                                start=(i == 0 and j == 0),
                                stop=stop,
                            )
                        # build plane i+1 during group 0's round i, two
                        # chunks per tap, so it's ready when round i+1 (and
                        # every later group) reads it
                        if g == 0 and i < K - 1:
                            for ci in range(2 * j, min(2 * j + 2, len(chunks))):
                                emit_shift_chunk(i + 1, ci)
                for c in range(nch):
                    gwc = slice(
                        g0 * W + c * CHUNK, g0 * W + (c + 1) * CHUNK
                    )
                    otc = o_pool.tile(
                        [H, CHUNK], bf16, tag="ot", name=f"ot{g}_{c}"
                    )
                    nc.scalar.copy(otc[:], pss[c][:])
                    outs_pending.append((gwc, otc))
                if last_g:
                    # all out DMAs issue on the sync ring after the last
                    # group's loads: issued earlier they would interleave
                    # into the input stream and push the last input (and the
                    # whole tail chain) later, while the post-input DMA
                    # window sat idle
                    for gwc, otc in outs_pending:
                        nc.sync.dma_start(out_d[:, gwc], otc[:])

    nc.compile()
    return nc


def get_nc(debug=False):
    key = ("nc", debug)
    if key not in _CACHE:
        _CACHE[key] = _build_module(debug=debug)
    return _CACHE[key]


def prep_inputs(data, core):
    """Full inputs -> list of per-core input dicts (host-side shard + pad)."""
    data = np.asarray(data, dtype=np.float32)
    core = np.asarray(core, dtype=np.float32)
    # [b, i, j, c, h, w] -> [i, j, h, b, c, w]: k-slices i-major on device
    core_t = np.ascontiguousarray(
        core.reshape(B, K, K, C, H, W).transpose(1, 2, 4, 0, 3, 5).astype(BF16)
    )
    dp = np.zeros((HP, B, C, WP), BF16)
    dp[PAD : PAD + H, :, :, PAD : PAD + W] = data.transpose(2, 0, 1, 3)
    ident = np.ascontiguousarray(np.eye(128, dtype=BF16))
    # S_i shifts rows up by i via lhsT.T @ x: S_i[p, m] = 1 iff p == m + i;
    # F_i patches rows m >= 128 - i from the 4-row bottom tile
    smat = np.zeros((128, (K - 1) * 128), BF16)
    fmat = np.zeros((2 * PAD, (K - 1) * 128), BF16)
    for i in range(1, K):
        smat[:, (i - 1) * 128 : i * 128] = np.eye(128, k=-i, dtype=BF16)
        for p in range(2 * PAD):
            m = 128 + p - i
            if 0 <= m < 128:
                fmat[p, (i - 1) * 128 + m] = 1.0
    in_maps = []
    for r in range(NCORES):
        cs = slice(r * CPC, (r + 1) * CPC)
        core_r = np.ascontiguousarray(core_t[:, :, :, :, cs, :]).reshape(
            KK, H, NIMG * W
        )
        data_r = np.ascontiguousarray(dp[:, :, cs, :]).reshape(HP, NIMG * WP)
        in_maps.append(
            {
                "core": core_r,
                "data": data_r,
                "ident": ident,
                "smat": smat,
                "fmat": fmat,
            }
        )
    return in_maps


def assemble(per_core_outs):
    """Per-core 'out' arrays [H, NIMG*W] (bf16) -> full [B, C, H, W] f32."""
    out = np.empty((B, C, H, W), np.float32)
    for r, o in enumerate(per_core_outs):
        cs = slice(r * CPC, (r + 1) * CPC)
        out[:, cs] = (
            np.asarray(o).astype(np.float32)
            .reshape(H, B, CPC, W)
            .transpose(1, 2, 0, 3)
        )
    return out


def run_spmd(in_maps, trace=False, trace_cores=None):
    from concourse.bass_utils import run_bass_kernel_spmd

    return run_bass_kernel_spmd(
        get_nc(),
        in_maps,
        list(range(NCORES)),
        trace=trace,
        trace_cores=trace_cores,
    )


def kernel(data, core):
    res = run_spmd(prep_inputs(data, core))
    return assemble([res.results[r]["out"] for r in range(NCORES)])


# revision 26
# speedup vs baseline: 2.2534x; 1.0478x over previous
"""Per-pixel dynamic 5x5 conv (KernelConv) on 8 Trainium2 NeuronCores.

out[b,c,h,w] = sum_{i,j} core[b,(i*5+j)*C+c,h,w] * pad(data)[b,c,h+i,w+j]

Sharding: channel groups of 8 per core (x 4 batches = 32 channel-images/core).
Layout on chip: partitions = h (128), free dim = image-blocks x w.

All HBM traffic is bf16 (host converts): halves the DMA bytes vs f32, and the
2e-2 rel-err budget dwarfs bf16 rounding (~0.5% worst case here).

The DMA roofline (360 GB/s shared across all queues in the cost model) is the
binding constraint, so the kernel minimizes bytes moved:
- core: 26.2 MB/core, read once as 100 per-k group slices.
- data: loaded ONCE (padded rows 0..127 full-width + a 4-row bottom tile).
  The other four row-shifted copies the taps need are built ON CHIP by the
  TensorEngine: shifted-identity matmuls (S_i = eye shifted by i, plus a
  4-row fixup from the bottom tile) into PSUM, ACT-copied back to bf16 SBUF
  512-col chunks at a time. This replaces ~4.3 MB of duplicate HBM reads.
  (Row shifts cannot be done with plain engine copies: SBUF access patterns
  may only start at partitions 0/32/64/96; only PE matmul or DMA cross
  partitions, and in the cost model DMA bandwidth is the bottleneck.)
- out: written as bf16, per 512-col PSUM bank.

Compute structure (all-bf16 operands keep the DVE 2x perf mode; a mixed f32
output would forfeit it):
- DVE does only the 25 per-k products per group ([128,1024] each).
- The 24 adds per pixel run on the TensorEngine as identity matmuls
  accumulating into PSUM (f32, exact): 2 bank-matmuls per product. Taps run
  i-major so plane i+1 is PE-built (interleaved into round i's tap stream)
  just before round i+1 consumes it.
- ACT drains each finished PSUM bank to bf16; all out DMAs are issued on the
  sync ring after the last group's loads so their transfers fill the
  post-input DMA window instead of delaying the last input.
- 4 groups of 8 images: PSUM = 4 accum banks (reused across groups) + 4
  shift-chunk banks; the small last group keeps the final
  mul->matmul->drain->out chain short, and 728ns slice DMAs still outpace
  the 565ns per-DMA ring issue rate.
"""

import numpy as np
import ml_dtypes

B, C, H, W = 4, 64, 128, 128
K, PAD, KK = 5, 2, 25
NCORES = 8
CPC = C // NCORES            # channels per core = 8
NIMG = B * CPC               # channel-images per core = 32
GRPS = (20, 12)              # images per compute group: first is wide so PE
                             # fits its accum matmuls plus all plane builds in
                             # the window (~92% busy); the second's 3 PSUM
                             # banks serve as shift-chunk transients first
NG = len(GRPS)
WP = W + 2 * PAD             # 132
HP = H + 2 * PAD             # 132
CHUNK = 512                  # PSUM bank = 512 f32 per partition
FW = NIMG * WP               # full padded width = 4224

BF16 = ml_dtypes.bfloat16

_CACHE = {}


def _shift_chunks():
    out = []
    off = 0
    while off < FW:
        n = min(CHUNK, FW - off)
        out.append((off, n))
        off += n
    return out


def _build_module(debug=False):
    import concourse.tile as tile
    from concourse import bacc, bass, mybir

    f32 = mybir.dt.float32
    bf16 = mybir.dt.bfloat16
    nc = bacc.Bacc(
        "TRN2", target_bir_lowering=False, debug=debug, num_devices=NCORES
    )
    core_d = nc.dram_tensor(
        "core", [KK, H, NIMG * W], bf16, kind="ExternalInput"
    ).ap()
    data_d = nc.dram_tensor(
        "data", [HP, FW], bf16, kind="ExternalInput"
    ).ap()
    ident_d = nc.dram_tensor("ident", [128, 128], bf16, kind="ExternalInput").ap()
    smat_d = nc.dram_tensor(
        "smat", [128, (K - 1) * 128], bf16, kind="ExternalInput"
    ).ap()
    fmat_d = nc.dram_tensor(
        "fmat", [2 * PAD, (K - 1) * 128], bf16, kind="ExternalInput"
    ).ap()
    out_d = nc.dram_tensor("out", [H, NIMG * W], bf16, kind="ExternalOutput").ap()

    chunks = _shift_chunks()

    with tile.TileContext(nc) as tc:
        with (
            tc.tile_pool(name="constp", bufs=1) as k_pool,
            tc.tile_pool(name="shifts", bufs=1) as sh_pool,
            tc.tile_pool(name="corep", bufs=8) as ca_pool,
            tc.tile_pool(name="corepb", bufs=10) as cb_pool,
            tc.tile_pool(name="prodp", bufs=8) as pa_pool,
            tc.tile_pool(name="prodpb", bufs=8) as pb_pool,
            tc.tile_pool(name="outp", bufs=NG * 2) as o_pool,
            tc.tile_pool(
                name="psump", bufs=4, space=bass.MemorySpace.PSUM
            ) as ps_pool,
            tc.tile_pool(
                name="pshp", bufs=3, space=bass.MemorySpace.PSUM
            ) as psh_pool,
        ):
            ident = k_pool.tile([128, 128], bf16, tag="ident")
            nc.scalar.dma_start(ident[:], ident_d[:, :])
            smat = k_pool.tile([128, (K - 1) * 128], bf16, tag="smat")
            nc.scalar.dma_start(smat[:], smat_d[:, :])
            fmat = k_pool.tile([2 * PAD, (K - 1) * 128], bf16, tag="fmat")
            nc.scalar.dma_start(fmat[:], fmat_d[:, :])
            dpb = k_pool.tile([2 * PAD, FW], bf16, tag="dpb")
            nc.scalar.dma_start(dpb[:], data_d[H:HP, :])
            # full-width shift planes, one tile per row shift i so a tap's
            # product waits only on its own plane's writers
            shp = []
            for i in range(K):
                t = sh_pool.tile([H, FW], bf16, tag=f"shp{i}", name=f"shp{i}")
                shp.append(t)
            nc.scalar.dma_start(shp[0][:], data_d[0:H, :])
            nchunks = len(chunks)

            def emit_shift_chunk(i, ci):
                off, n = chunks[ci]
                psh = psh_pool.tile(
                    [H, CHUNK], f32, tag="psh", name=f"psh{i}_{ci}"
                )
                nc.tensor.matmul(
                    psh[:, :n],
                    smat[:, (i - 1) * 128 : i * 128],
                    shp[0][:, off : off + n],
                    start=True,
                    stop=False,
                )
                nc.tensor.matmul(
                    psh[:, :n],
                    fmat[:, (i - 1) * 128 : i * 128],
                    dpb[:, off : off + n],
                    start=False,
                    stop=True,
                )
                nc.scalar.copy(shp[i][:, off : off + n], psh[:, :n])

            # plane 1 is pre-built while PE is otherwise idle; planes 2-4
            # stream at ~1.3 chunks per tap, each finishing comfortably
            # before group 0's round that consumes it (bulk-prebuilding more
            # planes front-loads the in-order PE queue and the resulting
            # pipeline lag makes plane 4 miss round 4 instead)
            for ci in range(nchunks):
                emit_shift_chunk(1, ci)
            sched = {}
            t = 0
            for p in (2, 3, 4):
                per = [2, 2, 1, 1, 1, 1, 1] if p == 2 else [1] * 9
                ci = 0
                for n in per:
                    sched.setdefault(t, []).extend(
                        (p, ci + d) for d in range(n)
                    )
                    ci += n
                    t += 1
                t = {2: 7, 3: 16}.get(p, t)
            outs_pending = []
            for g, grp in enumerate(GRPS):
                g0 = sum(GRPS[:g])
                gw_cols = grp * W
                nch = gw_cols // CHUNK
                gw = slice(g0 * W, g0 * W + gw_cols)
                last_g = g == NG - 1
                # g1's accum banks come from the shift-chunk pool (dead
                # after plane building) so g1 never waits on g0's drains
                apool = psh_pool if g == NG - 1 else ps_pool
                pss = [
                    apool.tile([H, CHUNK], f32, tag="ps" if g != 1 else "psh",
                               name=f"ps{g}_{c}")
                    for c in range(nch)
                ]
                for i in range(K):
                    for j in range(K):
                        cti = (ca_pool if g == 0 else cb_pool).tile(
                            [H, gw_cols], bf16, tag=f"ct{grp}",
                            name=f"ct_{g}_{i}_{j}",
                        )
                        nc.sync.dma_start(cti[:], core_d[i * K + j, :, gw])
                        pti = (pa_pool if g == 0 else pb_pool).tile(
                            [H, gw_cols], bf16, tag=f"pt{grp}",
                            name=f"pt_{g}_{i}_{j}",
                        )
                        shv = shp[i].rearrange("p (b w) -> p b w", b=NIMG)
                        nc.vector.tensor_mul(
                            pti.rearrange("p (b w) -> p b w", b=grp),
                            cti.rearrange("p (b w) -> p b w", b=grp),
                            shv[:, g0 : g0 + grp, j : j + W],
                        )
                        stop = i == K - 1 and j == K - 1
                        for c in range(nch):
                            nc.tensor.matmul(
                                pss[c][:],
                                ident[:],
                                pti[:, c * CHUNK : (c + 1) * CHUNK],
                                start=(i == 0 and j == 0),
                                stop=stop,
                            )
                        if g == 0:
                            for p, ci in sched.get(i * K + j, ()):
                                emit_shift_chunk(p, ci)
                for c in range(nch):
                    gwc = slice(
                        g0 * W + c * CHUNK, g0 * W + (c + 1) * CHUNK
                    )
                    otc = o_pool.tile(
                        [H, CHUNK], bf16, tag="ot", name=f"ot{g}_{c}"
                    )
                    if last_g and c % 2 == 1:
                        # split the final drains across ACT and the
                        # now-idle DVE so they don't serialize
                        nc.vector.tensor_copy(otc[:], pss[c][:])
                    else:
                        nc.scalar.copy(otc[:], pss[c][:])
                    outs_pending.append((gwc, otc))
                if last_g:
                    # hidden groups' out DMAs issue on the sync ring here,
                    # after the last group's loads: issued earlier they would
                    # interleave into the input stream and push the last
                    # input (and the whole tail chain) later, while the
                    # post-input DMA window sat idle. The last group's two
                    # outs ride separate rings to dodge issue serialization.
                    for gwc, otc in outs_pending[:-2]:
                        nc.sync.dma_start(out_d[:, gwc], otc[:])
                    nc.scalar.dma_start(*(
                        (out_d[:, outs_pending[-2][0]], outs_pending[-2][1][:])
                    ))
                    nc.sync.dma_start(out_d[:, outs_pending[-1][0]],
                                      outs_pending[-1][1][:])

    nc.compile()
    return nc


def get_nc(debug=False):
    key = ("nc", debug)
    if key not in _CACHE:
        _CACHE[key] = _build_module(debug=debug)
    return _CACHE[key]


def prep_inputs(data, core):
    """Full inputs -> list of per-core input dicts (host-side shard + pad)."""
    data = np.asarray(data, dtype=np.float32)
    core = np.asarray(core, dtype=np.float32)
    # [b, i, j, c, h, w] -> [i, j, h, b, c, w]: k-slices i-major on device
    core_t = np.ascontiguousarray(
        core.reshape(B, K, K, C, H, W).transpose(1, 2, 4, 0, 3, 5).astype(BF16)
    )
    dp = np.zeros((HP, B, C, WP), BF16)
    dp[PAD : PAD + H, :, :, PAD : PAD + W] = data.transpose(2, 0, 1, 3)
    ident = np.ascontiguousarray(np.eye(128, dtype=BF16))
    # S_i shifts rows up by i via lhsT.T @ x: S_i[p, m] = 1 iff p == m + i;
    # F_i patches rows m >= 128 - i from the 4-row bottom tile
    smat = np.zeros((128, (K - 1) * 128), BF16)
    fmat = np.zeros((2 * PAD, (K - 1) * 128), BF16)
    for i in range(1, K):
        smat[:, (i - 1) * 128 : i * 128] = np.eye(128, k=-i, dtype=BF16)
        for p in range(2 * PAD):
            m = 128 + p - i
            if 0 <= m < 128:
                fmat[p, (i - 1) * 128 + m] = 1.0
    in_maps = []
    for r in range(NCORES):
        cs = slice(r * CPC, (r + 1) * CPC)
        core_r = np.ascontiguousarray(core_t[:, :, :, :, cs, :]).reshape(
            KK, H, NIMG * W
        )
        data_r = np.ascontiguousarray(dp[:, :, cs, :]).reshape(HP, NIMG * WP)
        in_maps.append(
            {
                "core": core_r,
                "data": data_r,
                "ident": ident,
                "smat": smat,
                "fmat": fmat,
            }
        )
    return in_maps


def assemble(per_core_outs):
    """Per-core 'out' arrays [H, NIMG*W] (bf16) -> full [B, C, H, W] f32."""
    out = np.empty((B, C, H, W), np.float32)
    for r, o in enumerate(per_core_outs):
        cs = slice(r * CPC, (r + 1) * CPC)
        out[:, cs] = (
            np.asarray(o).astype(np.float32)
            .reshape(H, B, CPC, W)
            .transpose(1, 2, 0, 3)
        )
    return out


def run_spmd(in_maps, trace=False, trace_cores=None):
    from concourse.bass_utils import run_bass_kernel_spmd

    return run_bass_kernel_spmd(
        get_nc(),
        in_maps,
        list(range(NCORES)),
        trace=trace,
        trace_cores=trace_cores,
    )


def kernel(data, core):
    res = run_spmd(prep_inputs(data, core))
    return assemble([res.results[r]["out"] for r in range(NCORES)])
